# revision 9
# baseline (speedup 1.0000x reference)
"""Anisotropic collisions kernel for 8 TRN2 NeuronCores.

Math: for each of 9*64*64 = 36864 independent systems (mode, spatial cell),
build tridiagonal coefficients from Rosenbluth cumulative integrals of
flm(v) along v (512 points), then solve the tridiagonal system along v.

Key structural facts exploited (validated numerically vs f64 Thomas):
  1. The collision coefficients u (c2-term) and w (c1-term) decay ~1/v^2;
     beyond v-index T0 the tridiagonal system is identity to ~1e-4 * x.
     The solve therefore runs only on the first T0 columns of each
     512-system ("head"); the tail passes through (x = y) via an in-place
     scatter of the head solution into the input tile followed by one
     contiguous output DMA. Only S1 = sum(y*v) needs the full row: one
     full-length ratio scan (E1) on DVE.
  2. Thomas without the cp refinement (cp = c/b) is accurate to ~3e-3.

Scheduling: input DMA rides the SP queue, output DMA the Pool queue
(transfers on different queues overlap in time). Scans + reciprocal are
DVE-only ops; every elementwise tensor_tensor runs on the Pool engine
(flat-rate ALU, otherwise idle); activations (scaled copies) run on ACT.
Scale factors are folded into host-precomputed profiles so no
tensor_scalar / scalar_tensor_tensor is needed (TensorScalarPtr is
DVE-only on this toolchain): the weighted scans emit -w/2 and -u/2
directly, and the il2*(2DV/v) diagonal term uses a per-group outer
product profile il2[p] * 4DV/v[f].

Toolchain notes: this walrus build accepts only ONE sync-wait per
instruction; multi-wait instructions are split into standalone
InstEventSemaphore waits in a post-pass.
"""

import numpy as np
from contextlib import ExitStack

import concourse.bass as bass
import concourse.tile as tile
import concourse.mybir as mybir
from concourse.bass_utils import run_bass_kernel_spmd

F32 = mybir.dt.float32

NX, NY, NV = 64, 64, 512
N_MODES = 9
DV = 0.015625
Y_DT = 1.0e-12
FOUR_PI = 4.0 * np.pi
KY = FOUR_PI * Y_DT / 3.0

N_CORES = 8
ROWS_TOTAL = N_MODES * NX * NY            # 36864
ROWS_PER_CORE = ROWS_TOTAL // N_CORES     # 4608
FUSE = 4                                  # systems per partition row
GROUP_ROWS = 128 * FUSE                   # 512 systems per group
N_GROUPS = ROWS_PER_CORE // GROUP_ROWS    # 9
FD = FUSE * NV                            # 2048
T0 = 16                                   # head length per system
HD = FUSE * T0

_V = (np.arange(NV, dtype=np.float64) + 1.0) * DV

# f32 const blob: resetv [FD], then reset1h, pw2kh, g1wh, g2wh [HD each]
CF_W = FD + 4 * HD


def _profiles():
    v = _V
    vh = v[:T0]
    g1 = 3.0 * v**2 - v**4 - 2.0 * v
    g2 = v**4 - v
    pwn = -KY / (2.0 * DV * v**3)         # wn' = -w/2  (0.5 folded in)
    pun = -KY / (DV * DV * v**2)          # un' = -u/2
    r1 = np.ones(NV)
    r1[1:] = v[:-1] / v[1:]
    r1[0] = 0.0                           # E1 reset at each system start
    r3 = np.ones(T0)
    r3[1:] = (vh[:-1] / vh[1:])**3
    r3[0] = 0.0
    r2 = np.ones(T0)
    r2[1:] = (vh[:-1] / vh[1:])**2
    r2[0] = 0.0
    return np.concatenate([
        np.tile(r1, FUSE),
        np.tile(r3, FUSE),
        np.tile(r2, FUSE),
        np.tile(0.5 * g1[:T0] * pwn[:T0], FUSE),
        np.tile(0.5 * g2[:T0] * pun[:T0], FUSE),
    ])


def _legalize_multiwait(nc):
    """Split instructions with >1 sync wait: keep one wait on the
    instruction, hoist the rest onto standalone InstEventSemaphore ops
    immediately before it on the same engine (this walrus accepts only one
    wait per instruction)."""
    n = [0]

    def fresh(engine, wait):
        n[0] += 1
        return mybir.InstEventSemaphore(
            name=f"mwsplit-{n[0]}",
            engine=engine,
            sync_info=mybir.SyncInfo(on_wait=[wait], on_update=[]),
        )

    for fn in nc.m.functions:
        for blk in fn.blocks:
            out = []
            for ins in blk.instructions:
                si = ins.sync_info
                if si is not None and si.on_wait is not None and len(si.on_wait) > 1:
                    waits = list(si.on_wait)
                    for w in waits[:-1]:
                        out.append(fresh(ins.engine, w))
                    si.on_wait = [waits[-1]]
                out.append(ins)
            blk.instructions[:] = out


def build_nc(n_groups=N_GROUPS, legalize=True):
    nc = bass.Bass()
    rows = n_groups * GROUP_ROWS
    y_in = nc.declare_dram_parameter("y", [rows, NV], F32, isOutput=False)
    cf_in = nc.declare_dram_parameter("cf", [128, CF_W], F32, isOutput=False)
    ilp_in = nc.declare_dram_parameter("ilp", [128, n_groups * HD], F32, isOutput=False)
    out_ext = nc.declare_dram_parameter("out", [rows, NV], F32, isOutput=True)

    MUL = mybir.AluOpType.mult
    ADD = mybir.AluOpType.add
    SUB = mybir.AluOpType.subtract
    COPY = mybir.ActivationFunctionType.Copy

    pw0 = float(-KY / (2.0 * DV * _V[0]**3))
    pu0 = float(-KY / (DV * DV * _V[0]**2))
    vlast = float(_V[-1])

    with ExitStack() as ctx:
        tc = ctx.enter_context(tile.TileContext(nc))
        cpool = ctx.enter_context(tc.tile_pool(name="consts", bufs=1))

        cf = cpool.tile([128, CF_W], F32, tag="cf")
        ch = CF_W // 2
        nc.sync.dma_start(cf[:, 0:ch], cf_in[:, 0:ch])
        nc.scalar.dma_start(cf[:, ch:CF_W], cf_in[:, ch:CF_W])
        ilp = cpool.tile([128, n_groups * HD], F32, tag="ilp")
        nc.gpsimd.dma_start(ilp[:, :], ilp_in[:, :])

        resetv = cf[:, 0:FD]
        reset1h = cf[:, FD:FD + HD]
        pw2kh = cf[:, FD + HD:FD + 2 * HD]
        g1wh = cf[:, FD + 2 * HD:FD + 3 * HD]
        g2wh = cf[:, FD + 3 * HD:FD + 4 * HD]

        twos = cpool.tile([128, HD], F32, tag="twos")
        nc.gpsimd.memset(twos[:, :], 2.0)

        # touch consts so the tile framework orders compute after the loads
        for nm, seg in (("tc_f", cf), ("tc_i", ilp)):
            tch = cpool.tile([128, 1], F32, tag=nm)
            nc.vector.tensor_copy(out=tch[:, :], in_=seg[:, 0:1])

        io = ctx.enter_context(tc.tile_pool(name="io", bufs=6))
        e1p = ctx.enter_context(tc.tile_pool(name="e1", bufs=2))
        wk = ctx.enter_context(tc.tile_pool(name="work", bufs=3))

        # --- 3-stage software pipeline -------------------------------
        # A(g): input DMA, E1 scan, S1 seeds, head compaction, weighted
        #       scans wn/un, t3.
        # B(g): diagonal/off-diagonal assembly, reciprocal, alpha/mcp/beta.
        # C(g): dp/xb solve scans, scatter, output DMA.
        # Issuing A(g), C(g-2), B(g-1) keeps every in-order engine queue
        # stocked with ready work (the g-2 solve depends only on stage-B
        # results from the previous iteration).
        st = {}

        def stage_a(g):
            rsl = slice(g * GROUP_ROWS, (g + 1) * GROUP_ROWS)
            y_src = y_in[rsl, :].rearrange("(p j) v -> p (j v)", p=128)
            d = {}
            y4 = io.tile([128, FD], F32, tag="y4")
            if g == 0:
                h = FD // 2
                nc.sync.dma_start(y4[:, 0:h], y_src[:, 0:h])
                nc.scalar.dma_start(y4[:, h:FD], y_src[:, h:FD])
            else:
                nc.sync.dma_start(y4[:, :], y_src)
            d["y4"] = y4
            y4v = y4[:, :].rearrange("p (j v) -> p j v", j=FUSE)
            d["y4v"] = y4v

            # S1 per system via full-row ratio scan (E1_t = P(y*v)_t / v_t)
            E1 = e1p.tile([128, FD], F32, tag="E1")
            nc.vector.tensor_tensor_scan(E1[:, :], resetv, y4[:, :], 0.0,
                                         op0=MUL, op1=ADD)
            s1x2 = wk.tile([128, FUSE], F32, tag="s1x2")
            nc.scalar.activation(s1x2[:, :], E1[:, NV - 1::NV], COPY,
                                 bias=0.0, scale=pw0 * vlast)
            s1xp = wk.tile([128, FUSE], F32, tag="s1xp")
            nc.scalar.activation(s1xp[:, :], E1[:, NV - 1::NV], COPY,
                                 bias=0.0, scale=0.5 * pu0 * vlast)

            yh = wk.tile([128, HD], F32, tag="yh")
            yhv = yh[:, :].rearrange("p (j v) -> p j v", j=FUSE)
            nc.gpsimd.tensor_copy(out=yhv[:, :, :], in_=y4v[:, :, 0:T0])
            d["yh"] = yh

            wg1 = wk.tile([128, HD], F32, tag="wg1")
            nc.gpsimd.tensor_tensor(out=wg1[:, :], in0=yh[:, :], in1=g1wh, op=MUL)
            wg2 = wk.tile([128, HD], F32, tag="wg2")
            nc.gpsimd.tensor_tensor(out=wg2[:, :], in0=yh[:, :], in1=g2wh, op=MUL)
            nc.gpsimd.tensor_tensor(out=wg1[:, 0::T0], in0=wg1[:, 0::T0],
                                    in1=s1x2[:, :], op=ADD)
            nc.gpsimd.tensor_tensor(out=wg2[:, 0::T0], in0=wg2[:, 0::T0],
                                    in1=s1xp[:, :], op=ADD)

            wn = wk.tile([128, HD], F32, tag="wn")
            nc.vector.tensor_tensor_scan(wn[:, :], reset1h, wg1[:, :], 0.0,
                                         op0=MUL, op1=ADD)
            un = wk.tile([128, HD], F32, tag="un")
            nc.vector.tensor_tensor_scan(un[:, :], pw2kh, wg2[:, :], 0.0,
                                         op0=MUL, op1=ADD)
            d["wn"], d["un"] = wn, un

            t3 = wk.tile([128, HD], F32, tag="t3")
            nc.scalar.activation(t3[:, :], yh[:, :], COPY,
                                 bias=1.0, scale=float(8.0 * np.pi * Y_DT))
            d["t3"] = t3
            return d

        def stage_b(g, d):
            wn, un, yh, t3 = d["wn"], d["un"], d["yh"], d["t3"]
            b1 = wk.tile([128, HD], F32, tag="b1")
            nc.gpsimd.tensor_tensor(out=b1[:, :], in0=t3[:, :], in1=un[:, :], op=SUB)
            bil = wk.tile([128, HD], F32, tag="bil")
            nc.gpsimd.tensor_tensor(out=bil[:, :], in0=wn[:, :],
                                    in1=ilp[:, g * HD:(g + 1) * HD], op=MUL)
            bpos = wk.tile([128, HD], F32, tag="bpos")
            nc.gpsimd.tensor_tensor(out=bpos[:, :], in0=b1[:, :], in1=bil[:, :], op=ADD)
            binv = wk.tile([128, HD], F32, tag="binv")
            nc.vector.reciprocal(out=binv[:, :], in_=bpos[:, :])
            binv2 = wk.tile([128, HD], F32, tag="binv2")
            nc.gpsimd.tensor_tensor(out=binv2[:, :], in0=binv[:, :], in1=twos[:, :], op=MUL)

            a_n = wk.tile([128, HD], F32, tag="a_n")
            nc.gpsimd.tensor_tensor(out=a_n[:, :], in0=un[:, :], in1=wn[:, :], op=SUB)
            c_n = wk.tile([128, HD], F32, tag="c_n")
            nc.gpsimd.tensor_tensor(out=c_n[:, :], in0=un[:, :], in1=wn[:, :], op=ADD)

            alpha = wk.tile([128, HD], F32, tag="alpha")    # -a/b
            nc.gpsimd.tensor_tensor(out=alpha[:, :], in0=a_n[:, :], in1=binv2[:, :], op=MUL)
            av = alpha[:, :].rearrange("p (j v) -> p j v", j=FUSE)
            nc.gpsimd.memset(av[:, :, 0:1], 0.0)            # fwd scan reset
            mcp = wk.tile([128, HD], F32, tag="mcp")        # -c/b
            nc.gpsimd.tensor_tensor(out=mcp[:, :], in0=c_n[:, :], in1=binv2[:, :], op=MUL)
            mv = mcp[:, :].rearrange("p (j v) -> p j v", j=FUSE)
            nc.gpsimd.memset(mv[:, :, T0 - 1:T0], 0.0)      # bwd scan reset
            beta = wk.tile([128, HD], F32, tag="beta")      # y/b
            nc.gpsimd.tensor_tensor(out=beta[:, :], in0=yh[:, :], in1=binv[:, :], op=MUL)
            d["alpha"], d["mcp"], d["beta"] = alpha, mcp, beta

        def stage_c(g, d):
            rsl = slice(g * GROUP_ROWS, (g + 1) * GROUP_ROWS)
            x_dst = out_ext[rsl, :].rearrange("(p j) v -> p (j v)", p=128)
            alpha, mcp, beta = d["alpha"], d["mcp"], d["beta"]
            dp = wk.tile([128, HD], F32, tag="dp")
            nc.vector.tensor_tensor_scan(dp[:, :], alpha[:, :], beta[:, :], 0.0,
                                         op0=MUL, op1=ADD)
            xh = wk.tile([128, HD], F32, tag="xh")
            nc.vector.tensor_tensor_scan(xh[:, ::-1], mcp[:, ::-1], dp[:, ::-1], 0.0,
                                         op0=MUL, op1=ADD)
            xhv = xh[:, :].rearrange("p (j v) -> p j v", j=FUSE)
            nc.gpsimd.tensor_copy(out=d["y4v"][:, :, 0:T0], in_=xhv[:, :, :])
            if g >= n_groups - 2:
                h = FD // 2
                nc.sync.dma_start(x_dst[:, 0:h], d["y4"][:, 0:h])
                eng = nc.scalar if g % 2 == 0 else nc.gpsimd
                eng.dma_start(x_dst[:, h:FD], d["y4"][:, h:FD])
            elif g % 2 == 0:
                nc.scalar.dma_start(x_dst, d["y4"][:, :])
            else:
                nc.gpsimd.dma_start(x_dst, d["y4"][:, :])

        for g in range(n_groups + 2):
            if g < n_groups:
                st[g] = stage_a(g)
            if g - 2 >= 0:
                stage_c(g - 2, st[g - 2])
                del st[g - 2]
            if g - 1 < n_groups and g - 1 >= 0:
                stage_b(g - 1, st[g - 1])

    if legalize:
        _legalize_multiwait(nc)
    return nc


_NC_CACHE = {}


def _get_nc(n_groups=N_GROUPS):
    if n_groups not in _NC_CACHE:
        _NC_CACHE[n_groups] = build_nc(n_groups)
    return _NC_CACHE[n_groups]


_CF_CACHE = None


def make_inputs(y_shard, il2_rows, n_groups=N_GROUPS):
    """Per-core input map. y_shard [rows, 512] f32; il2_rows [rows] f32
    (holding il*(il+1)/2 per row)."""
    global _CF_CACHE
    if _CF_CACHE is None:
        _CF_CACHE = np.broadcast_to(_profiles()[None, :], (128, CF_W)
                                    ).astype(np.float32).copy()
    il2 = il2_rows.reshape(n_groups, 128, FUSE)[:, :, 0]          # [g, 128]
    prof = (4.0 * DV / _V[:T0]).astype(np.float64)                # [T0]
    ilp = (il2[:, :, None, None] * prof[None, None, None, :])     # [g,128,1,T0]
    ilp = np.broadcast_to(ilp, (n_groups, 128, FUSE, T0))
    ilp = ilp.transpose(1, 0, 2, 3).reshape(128, n_groups * HD).astype(np.float32)
    return {
        "y": np.ascontiguousarray(y_shard, dtype=np.float32),
        "cf": _CF_CACHE,
        "ilp": np.ascontiguousarray(ilp),
    }


def kernel(y, il_arr):
    y = np.asarray(y, dtype=np.float32)
    il_arr = np.asarray(il_arr)
    yf = y.reshape(ROWS_TOTAL, NV)
    il_f = il_arr.astype(np.float64)
    il2_all = np.repeat(il_f * (il_f + 1.0) / 2.0, NX * NY).astype(np.float32)

    nc = _get_nc()
    in_maps = []
    for c in range(N_CORES):
        rs = slice(c * ROWS_PER_CORE, (c + 1) * ROWS_PER_CORE)
        in_maps.append(make_inputs(yf[rs], il2_all[rs]))
    res = run_bass_kernel_spmd(nc, in_maps, core_ids=list(range(N_CORES)))
    outs = [res.results[c]["out"] for c in range(N_CORES)]
    x = np.concatenate(outs, axis=0).reshape(N_MODES, NX, NY, NV)
    return x.astype(np.float32)


# revision 14
# speedup vs baseline: 1.1906x; 1.1906x over previous
"""Anisotropic collisions kernel for 8 TRN2 NeuronCores.

Math: for each of 9*64*64 = 36864 independent systems (mode, spatial cell),
build tridiagonal coefficients from Rosenbluth cumulative integrals of
flm(v) along v (512 points), then solve the tridiagonal system along v.

Key structural facts exploited (validated numerically vs f64 Thomas):
  1. The collision coefficients u (c2-term) and w (c1-term) decay ~1/v^2;
     beyond v-index T0 the tridiagonal system is identity to ~1e-4 * x.
     The solve therefore runs only on the first T0 columns of each
     512-system ("head"); the tail passes through (x = y) via an in-place
     scatter of the head solution into the input tile followed by one
     contiguous output DMA. Only S1 = sum(y*v) needs the full row: one
     full-length ratio scan (E1) on DVE.
  2. Thomas without the cp refinement (cp = c/b) is accurate to ~3e-3.

Scheduling: input DMA rides the SP queue, output DMA the Pool queue
(transfers on different queues overlap in time). Scans + reciprocal are
DVE-only ops; every elementwise tensor_tensor runs on the Pool engine
(flat-rate ALU, otherwise idle); activations (scaled copies) run on ACT.
Scale factors are folded into host-precomputed profiles so no
tensor_scalar / scalar_tensor_tensor is needed (TensorScalarPtr is
DVE-only on this toolchain): the weighted scans emit -w/2 and -u/2
directly, and the il2*(2DV/v) diagonal term uses a per-group outer
product profile il2[p] * 4DV/v[f].

Toolchain notes: this walrus build accepts only ONE sync-wait per
instruction; multi-wait instructions are split into standalone
InstEventSemaphore waits in a post-pass.
"""

import numpy as np
from contextlib import ExitStack

import concourse.bass as bass
import concourse.tile as tile
import concourse.mybir as mybir
from concourse.bass_utils import run_bass_kernel_spmd

F32 = mybir.dt.float32

NX, NY, NV = 64, 64, 512
N_MODES = 9
DV = 0.015625
Y_DT = 1.0e-12
FOUR_PI = 4.0 * np.pi
KY = FOUR_PI * Y_DT / 3.0

N_CORES = 8
ROWS_TOTAL = N_MODES * NX * NY            # 36864
ROWS_PER_CORE = ROWS_TOTAL // N_CORES     # 4608
FUSE = 4                                  # systems per partition row
GROUP_ROWS = 128 * FUSE                   # 512 systems per group
N_GROUPS = ROWS_PER_CORE // GROUP_ROWS    # 9
FD = FUSE * NV                            # 2048
T0 = 16                                   # head length per system
HD = FUSE * T0

_V = (np.arange(NV, dtype=np.float64) + 1.0) * DV

# f32 const blob: resetv [FD], then reset1h, pw2kh, g1wh, g2wh [HD each]
CF_W = FD + 4 * HD


def _profiles():
    v = _V
    vh = v[:T0]
    g1 = 3.0 * v**2 - v**4 - 2.0 * v
    g2 = v**4 - v
    pwn = -KY / (2.0 * DV * v**3)         # wn' = -w/2  (0.5 folded in)
    pun = -KY / (DV * DV * v**2)          # un' = -u/2
    r1 = np.ones(NV)
    r1[1:] = v[:-1] / v[1:]
    r1[0] = 0.0                           # E1 reset at each system start
    r3 = np.ones(T0)
    r3[1:] = (vh[:-1] / vh[1:])**3
    r3[0] = 0.0
    r2 = np.ones(T0)
    r2[1:] = (vh[:-1] / vh[1:])**2
    r2[0] = 0.0
    return np.concatenate([
        np.tile(r1, FUSE),
        np.tile(r3, FUSE),
        np.tile(r2, FUSE),
        np.tile(0.5 * g1[:T0] * pwn[:T0], FUSE),
        np.tile(0.5 * g2[:T0] * pun[:T0], FUSE),
    ])


def _legalize_multiwait(nc):
    """Split instructions with >1 sync wait: keep one wait on the
    instruction, hoist the rest onto standalone InstEventSemaphore ops
    immediately before it on the same engine (this walrus accepts only one
    wait per instruction)."""
    n = [0]

    def fresh(engine, wait):
        n[0] += 1
        return mybir.InstEventSemaphore(
            name=f"mwsplit-{n[0]}",
            engine=engine,
            sync_info=mybir.SyncInfo(on_wait=[wait], on_update=[]),
        )

    for fn in nc.m.functions:
        for blk in fn.blocks:
            out = []
            for ins in blk.instructions:
                si = ins.sync_info
                if si is not None and si.on_wait is not None and len(si.on_wait) > 1:
                    waits = list(si.on_wait)
                    for w in waits[:-1]:
                        out.append(fresh(ins.engine, w))
                    si.on_wait = [waits[-1]]
                out.append(ins)
            blk.instructions[:] = out


def build_nc(n_groups=N_GROUPS, legalize=True):
    nc = bass.Bass()
    rows = n_groups * GROUP_ROWS
    y_in = nc.declare_dram_parameter("y", [rows, NV], F32, isOutput=False)
    cf_in = nc.declare_dram_parameter("cf", [128, CF_W], F32, isOutput=False)
    ilp_in = nc.declare_dram_parameter("ilp", [128, n_groups * HD], F32, isOutput=False)
    out_ext = nc.declare_dram_parameter("out", [rows, NV], F32, isOutput=True)

    MUL = mybir.AluOpType.mult
    ADD = mybir.AluOpType.add
    SUB = mybir.AluOpType.subtract
    COPY = mybir.ActivationFunctionType.Copy

    pw0 = float(-KY / (2.0 * DV * _V[0]**3))
    pu0 = float(-KY / (DV * DV * _V[0]**2))
    vlast = float(_V[-1])

    with ExitStack() as ctx:
        tc = ctx.enter_context(tile.TileContext(nc))
        cpool = ctx.enter_context(tc.tile_pool(name="consts", bufs=1))

        cf = cpool.tile([128, CF_W], F32, tag="cf")
        nc.gpsimd.dma_start(cf[:, :], cf_in[:, :])
        ilp = cpool.tile([128, n_groups * HD], F32, tag="ilp")
        nc.gpsimd.dma_start(ilp[:, :], ilp_in[:, :])

        resetv = cf[:, 0:FD]
        reset1h = cf[:, FD:FD + HD]
        pw2kh = cf[:, FD + HD:FD + 2 * HD]
        g1wh = cf[:, FD + 2 * HD:FD + 3 * HD]
        g2wh = cf[:, FD + 3 * HD:FD + 4 * HD]

        twos = cpool.tile([128, HD], F32, tag="twos")
        nc.gpsimd.memset(twos[:, :], 2.0)

        # touch consts so the tile framework orders compute after the loads
        for nm, seg in (("tc_f", cf), ("tc_i", ilp)):
            tch = cpool.tile([128, 1], F32, tag=nm)
            nc.vector.tensor_copy(out=tch[:, :], in_=seg[:, 0:1])

        io = ctx.enter_context(tc.tile_pool(name="io", bufs=9))
        e1p = ctx.enter_context(tc.tile_pool(name="e1", bufs=3))
        wk = ctx.enter_context(tc.tile_pool(name="work", bufs=4))

        # --- 3-stage software pipeline -------------------------------
        # A(g): input DMA, E1 scan, S1 seeds, head compaction, weighted
        #       scans wn/un, t3.
        # B(g): diagonal/off-diagonal assembly, reciprocal, alpha/mcp/beta.
        # C(g): dp/xb solve scans, scatter, output DMA.
        # Issuing A(g), C(g-2), B(g-1) keeps every in-order engine queue
        # stocked with ready work (the g-2 solve depends only on stage-B
        # results from the previous iteration).
        st = {}

        def stage_a(g):
            rsl = slice(g * GROUP_ROWS, (g + 1) * GROUP_ROWS)
            y_src = y_in[rsl, :].rearrange("(p j) v -> p (j v)", p=128)
            x3d = out_ext[rsl, :].rearrange("(p j) v -> p j v", p=128)
            d = {"x3d": x3d}
            y4 = io.tile([128, FD], F32, tag="y4")
            (nc.scalar if g == 4 else nc.sync).dma_start(y4[:, :], y_src)
            d["y4"] = y4
            y4v = y4[:, :].rearrange("p (j v) -> p j v", j=FUSE)
            d["y4v"] = y4v

            # S1 per system via full-row ratio scan (E1_t = P(y*v)_t / v_t)
            E1 = e1p.tile([128, FD], F32, tag="E1")
            nc.vector.tensor_tensor_scan(E1[:, :], resetv, y4[:, :], 0.0,
                                         op0=MUL, op1=ADD)
            s1x2 = wk.tile([128, FUSE], F32, tag="s1x2")
            nc.scalar.activation(s1x2[:, :], E1[:, NV - 1::NV], COPY,
                                 bias=0.0, scale=pw0 * vlast)
            s1xp = wk.tile([128, FUSE], F32, tag="s1xp")
            nc.scalar.activation(s1xp[:, :], E1[:, NV - 1::NV], COPY,
                                 bias=0.0, scale=0.5 * pu0 * vlast)

            yh = wk.tile([128, HD], F32, tag="yh")
            yhv = yh[:, :].rearrange("p (j v) -> p j v", j=FUSE)
            nc.gpsimd.tensor_copy(out=yhv[:, :, :], in_=y4v[:, :, 0:T0])
            d["yh"] = yh
            # tail passthrough: x = y beyond T0 - stream it out as soon as
            # the input tile is resident (frees y4 after stage A)
            teng = nc.scalar if g % 2 == 0 else nc.gpsimd
            teng.dma_start(x3d[:, :, T0:NV], y4v[:, :, T0:NV])

            wg1 = wk.tile([128, HD], F32, tag="wg1")
            nc.gpsimd.tensor_tensor(out=wg1[:, :], in0=yh[:, :], in1=g1wh, op=MUL)
            wg2 = wk.tile([128, HD], F32, tag="wg2")
            nc.gpsimd.tensor_tensor(out=wg2[:, :], in0=yh[:, :], in1=g2wh, op=MUL)
            nc.gpsimd.tensor_tensor(out=wg1[:, 0::T0], in0=wg1[:, 0::T0],
                                    in1=s1x2[:, :], op=ADD)
            nc.gpsimd.tensor_tensor(out=wg2[:, 0::T0], in0=wg2[:, 0::T0],
                                    in1=s1xp[:, :], op=ADD)

            wn = wk.tile([128, HD], F32, tag="wn")
            nc.vector.tensor_tensor_scan(wn[:, :], reset1h, wg1[:, :], 0.0,
                                         op0=MUL, op1=ADD)
            un = wk.tile([128, HD], F32, tag="un")
            nc.vector.tensor_tensor_scan(un[:, :], pw2kh, wg2[:, :], 0.0,
                                         op0=MUL, op1=ADD)
            d["wn"], d["un"] = wn, un

            t3 = wk.tile([128, HD], F32, tag="t3")
            nc.scalar.activation(t3[:, :], yh[:, :], COPY,
                                 bias=1.0, scale=float(8.0 * np.pi * Y_DT))
            d["t3"] = t3
            return d

        def stage_b(g, d):
            wn, un, yh, t3 = d["wn"], d["un"], d["yh"], d["t3"]
            b1 = wk.tile([128, HD], F32, tag="b1")
            nc.gpsimd.tensor_tensor(out=b1[:, :], in0=t3[:, :], in1=un[:, :], op=SUB)
            bil = wk.tile([128, HD], F32, tag="bil")
            nc.gpsimd.tensor_tensor(out=bil[:, :], in0=wn[:, :],
                                    in1=ilp[:, g * HD:(g + 1) * HD], op=MUL)
            bpos = wk.tile([128, HD], F32, tag="bpos")
            nc.gpsimd.tensor_tensor(out=bpos[:, :], in0=b1[:, :], in1=bil[:, :], op=ADD)
            binv = wk.tile([128, HD], F32, tag="binv")
            nc.vector.reciprocal(out=binv[:, :], in_=bpos[:, :])
            binv2 = wk.tile([128, HD], F32, tag="binv2")
            nc.gpsimd.tensor_tensor(out=binv2[:, :], in0=binv[:, :], in1=twos[:, :], op=MUL)

            a_n = wk.tile([128, HD], F32, tag="a_n")
            nc.gpsimd.tensor_tensor(out=a_n[:, :], in0=un[:, :], in1=wn[:, :], op=SUB)
            c_n = wk.tile([128, HD], F32, tag="c_n")
            nc.gpsimd.tensor_tensor(out=c_n[:, :], in0=un[:, :], in1=wn[:, :], op=ADD)

            alpha = wk.tile([128, HD], F32, tag="alpha")    # -a/b
            nc.gpsimd.tensor_tensor(out=alpha[:, :], in0=a_n[:, :], in1=binv2[:, :], op=MUL)
            av = alpha[:, :].rearrange("p (j v) -> p j v", j=FUSE)
            nc.gpsimd.memset(av[:, :, 0:1], 0.0)            # fwd scan reset
            mcp = wk.tile([128, HD], F32, tag="mcp")        # -c/b
            nc.gpsimd.tensor_tensor(out=mcp[:, :], in0=c_n[:, :], in1=binv2[:, :], op=MUL)
            mv = mcp[:, :].rearrange("p (j v) -> p j v", j=FUSE)
            nc.gpsimd.memset(mv[:, :, T0 - 1:T0], 0.0)      # bwd scan reset
            beta = wk.tile([128, HD], F32, tag="beta")      # y/b
            nc.gpsimd.tensor_tensor(out=beta[:, :], in0=yh[:, :], in1=binv[:, :], op=MUL)
            d["alpha"], d["mcp"], d["beta"] = alpha, mcp, beta

        def stage_c(g, d):
            alpha, mcp, beta = d["alpha"], d["mcp"], d["beta"]
            dp = wk.tile([128, HD], F32, tag="dp")
            nc.vector.tensor_tensor_scan(dp[:, :], alpha[:, :], beta[:, :], 0.0,
                                         op0=MUL, op1=ADD)
            xh = wk.tile([128, HD], F32, tag="xh")
            nc.vector.tensor_tensor_scan(xh[:, ::-1], mcp[:, ::-1], dp[:, ::-1], 0.0,
                                         op0=MUL, op1=ADD)
            xhv = xh[:, :].rearrange("p (j v) -> p j v", j=FUSE)
            heng = nc.gpsimd if g % 2 == 0 else nc.scalar
            heng.dma_start(d["x3d"][:, :, 0:T0], xhv[:, :, :])

        for g in range(n_groups + 2):
            if g < n_groups:
                st[g] = stage_a(g)
            if g - 2 >= 0:
                stage_c(g - 2, st[g - 2])
                del st[g - 2]
            if g - 1 < n_groups and g - 1 >= 0:
                stage_b(g - 1, st[g - 1])

    if legalize:
        _legalize_multiwait(nc)
    return nc


_NC_CACHE = {}


def _get_nc(n_groups=N_GROUPS):
    if n_groups not in _NC_CACHE:
        _NC_CACHE[n_groups] = build_nc(n_groups)
    return _NC_CACHE[n_groups]


_CF_CACHE = None


def make_inputs(y_shard, il2_rows, n_groups=N_GROUPS):
    """Per-core input map. y_shard [rows, 512] f32; il2_rows [rows] f32
    (holding il*(il+1)/2 per row)."""
    global _CF_CACHE
    if _CF_CACHE is None:
        _CF_CACHE = np.broadcast_to(_profiles()[None, :], (128, CF_W)
                                    ).astype(np.float32).copy()
    il2 = il2_rows.reshape(n_groups, 128, FUSE)[:, :, 0]          # [g, 128]
    prof = (4.0 * DV / _V[:T0]).astype(np.float64)                # [T0]
    ilp = (il2[:, :, None, None] * prof[None, None, None, :])     # [g,128,1,T0]
    ilp = np.broadcast_to(ilp, (n_groups, 128, FUSE, T0))
    ilp = ilp.transpose(1, 0, 2, 3).reshape(128, n_groups * HD).astype(np.float32)
    return {
        "y": np.ascontiguousarray(y_shard, dtype=np.float32),
        "cf": _CF_CACHE,
        "ilp": np.ascontiguousarray(ilp),
    }


def kernel(y, il_arr):
    y = np.asarray(y, dtype=np.float32)
    il_arr = np.asarray(il_arr)
    yf = y.reshape(ROWS_TOTAL, NV)
    il_f = il_arr.astype(np.float64)
    il2_all = np.repeat(il_f * (il_f + 1.0) / 2.0, NX * NY).astype(np.float32)

    nc = _get_nc()
    in_maps = []
    for c in range(N_CORES):
        rs = slice(c * ROWS_PER_CORE, (c + 1) * ROWS_PER_CORE)
        in_maps.append(make_inputs(yf[rs], il2_all[rs]))
    res = run_bass_kernel_spmd(nc, in_maps, core_ids=list(range(N_CORES)))
    outs = [res.results[c]["out"] for c in range(N_CORES)]
    x = np.concatenate(outs, axis=0).reshape(N_MODES, NX, NY, NV)
    return x.astype(np.float32)


# revision 34
# speedup vs baseline: 1.2392x; 1.0408x over previous
"""Anisotropic collisions kernel for 8 TRN2 NeuronCores.

Math: for each of 9*64*64 = 36864 independent systems (mode, spatial cell),
build tridiagonal coefficients from Rosenbluth cumulative integrals of
flm(v) along v (512 points), then solve the tridiagonal system along v.

Key structural facts exploited (validated numerically vs f64 Thomas):
  1. The collision coefficients u (c2-term) and w (c1-term) decay ~1/v^2;
     beyond v-index T0 the tridiagonal system is identity to ~1e-4 * x.
     The solve therefore runs only on the first T0 columns of each
     512-system ("head"); the tail passes through (x = y) via an in-place
     scatter of the head solution into the input tile followed by one
     contiguous output DMA. Only S1 = sum(y*v) needs the full row: one
     full-length ratio scan (E1) on DVE.
  2. Thomas without the cp refinement (cp = c/b) is accurate to ~3e-3.

Scheduling: input DMA rides the SP queue, output DMA the Pool queue
(transfers on different queues overlap in time). Scans + reciprocal are
DVE-only ops; every elementwise tensor_tensor runs on the Pool engine
(flat-rate ALU, otherwise idle); activations (scaled copies) run on ACT.
Scale factors are folded into host-precomputed profiles so no
tensor_scalar / scalar_tensor_tensor is needed (TensorScalarPtr is
DVE-only on this toolchain): the weighted scans emit -w/2 and -u/2
directly, and the il2*(2DV/v) diagonal term uses a per-group outer
product profile il2[p] * 4DV/v[f].

Toolchain notes: this walrus build accepts only ONE sync-wait per
instruction; multi-wait instructions are split into standalone
InstEventSemaphore waits in a post-pass.
"""

import numpy as np
from contextlib import ExitStack

import concourse.bass as bass
import concourse.tile as tile
import concourse.mybir as mybir
from concourse.bass_utils import run_bass_kernel_spmd

F32 = mybir.dt.float32

NX, NY, NV = 64, 64, 512
N_MODES = 9
DV = 0.015625
Y_DT = 1.0e-12
FOUR_PI = 4.0 * np.pi
KY = FOUR_PI * Y_DT / 3.0

N_CORES = 8
ROWS_TOTAL = N_MODES * NX * NY            # 36864
ROWS_PER_CORE = ROWS_TOTAL // N_CORES     # 4608
FUSE = 4                                  # systems per partition row
GROUP_ROWS = 128 * FUSE                   # 512 systems per group
N_GROUPS = ROWS_PER_CORE // GROUP_ROWS    # 9
FD = FUSE * NV                            # 2048
T0 = 16                                   # head length per system
HD = FUSE * T0

_V = (np.arange(NV, dtype=np.float64) + 1.0) * DV

# f32 const blob: resetv [FD], reset1h/pw2kh/g1wh/g2wh [HD each], then
# pair-reduction profiles rpair [FD/2] and resetv2 [FD/2]
CF_W = FD + 4 * HD + FD
N_PAIR = 0                                # groups using the pair-reduced E1


def _profiles():
    v = _V
    vh = v[:T0]
    g1 = 3.0 * v**2 - v**4 - 2.0 * v
    g2 = v**4 - v
    pwn = -KY / (2.0 * DV * v**3)         # wn' = -w/2  (0.5 folded in)
    pun = -KY / (DV * DV * v**2)          # un' = -u/2
    r1 = np.ones(NV)
    r1[1:] = v[:-1] / v[1:]
    r1[0] = 0.0                           # E1 reset at each system start
    r3 = np.ones(T0)
    r3[1:] = (vh[:-1] / vh[1:])**3
    r3[0] = 0.0
    r2 = np.ones(T0)
    r2[1:] = (vh[:-1] / vh[1:])**2
    r2[0] = 0.0
    vp = v[1::2]                          # pair-reduced grid
    rpair = v[0::2] / v[1::2]
    r1p = np.ones(NV // 2)
    r1p[1:] = vp[:-1] / vp[1:]
    r1p[0] = 0.0
    return np.concatenate([
        np.tile(r1, FUSE),
        np.tile(r3, FUSE),
        np.tile(r2, FUSE),
        np.tile(0.5 * g1[:T0] * pwn[:T0], FUSE),
        np.tile(0.5 * g2[:T0] * pun[:T0], FUSE),
        np.tile(rpair, FUSE),
        np.tile(r1p, FUSE),
    ])


def _legalize_multiwait(nc):
    """Split instructions with >1 sync wait: keep one wait on the
    instruction, hoist the rest onto standalone InstEventSemaphore ops
    immediately before it on the same engine (this walrus accepts only one
    wait per instruction)."""
    n = [0]

    def fresh(engine, wait):
        n[0] += 1
        return mybir.InstEventSemaphore(
            name=f"mwsplit-{n[0]}",
            engine=engine,
            sync_info=mybir.SyncInfo(on_wait=[wait], on_update=[]),
        )

    for fn in nc.m.functions:
        for blk in fn.blocks:
            out = []
            for ins in blk.instructions:
                si = ins.sync_info
                if si is not None and si.on_wait is not None and len(si.on_wait) > 1:
                    waits = list(si.on_wait)
                    for w in waits[:-1]:
                        out.append(fresh(ins.engine, w))
                    si.on_wait = [waits[-1]]
                out.append(ins)
            blk.instructions[:] = out


def build_nc(n_groups=N_GROUPS, legalize=True):
    nc = bass.Bass()
    rows = n_groups * GROUP_ROWS
    y_in = nc.declare_dram_parameter("y", [rows, NV], F32, isOutput=False)
    cf_in = nc.declare_dram_parameter("cf", [128, CF_W], F32, isOutput=False)
    ilp_in = nc.declare_dram_parameter("ilp", [128, n_groups * HD], F32, isOutput=False)
    out_ext = nc.declare_dram_parameter("out", [rows, NV], F32, isOutput=True)

    MUL = mybir.AluOpType.mult
    ADD = mybir.AluOpType.add
    SUB = mybir.AluOpType.subtract
    COPY = mybir.ActivationFunctionType.Copy

    pw0 = float(-KY / (2.0 * DV * _V[0]**3))
    pu0 = float(-KY / (DV * DV * _V[0]**2))
    vlast = float(_V[-1])

    with ExitStack() as ctx:
        tc = ctx.enter_context(tile.TileContext(nc))
        cpool = ctx.enter_context(tc.tile_pool(name="consts", bufs=1))

        cf = cpool.tile([128, CF_W], F32, tag="cf")
        nc.gpsimd.dma_start(cf[:, 0:FD // 2], cf_in[:, 0:FD // 2])
        nc.scalar.dma_start(cf[:, FD // 2:FD], cf_in[:, FD // 2:FD])
        nc.gpsimd.dma_start(cf[:, FD:CF_W], cf_in[:, FD:CF_W])
        ilp = cpool.tile([128, n_groups * HD], F32, tag="ilp")
        nc.gpsimd.dma_start(ilp[:, :], ilp_in[:, :])

        resetv = cf[:, 0:FD]
        reset1h = cf[:, FD:FD + HD]
        pw2kh = cf[:, FD + HD:FD + 2 * HD]
        g1wh = cf[:, FD + 2 * HD:FD + 3 * HD]
        g2wh = cf[:, FD + 3 * HD:FD + 4 * HD]
        rpairc = cf[:, FD + 4 * HD:FD + 4 * HD + FD // 2]
        resetv2 = cf[:, FD + 4 * HD + FD // 2:FD + 4 * HD + FD]

        twos = cpool.tile([128, HD], F32, tag="twos")
        nc.gpsimd.memset(twos[:, :], 2.0)



        # touch consts so the tile framework orders compute after the loads
        for nm, seg in (("tc_f", cf), ("tc_i", ilp)):
            tch = cpool.tile([128, 1], F32, tag=nm)
            nc.vector.tensor_copy(out=tch[:, :], in_=seg[:, 0:1])

        io = ctx.enter_context(tc.tile_pool(name="io", bufs=9))
        e1p = ctx.enter_context(tc.tile_pool(name="e1", bufs=3))
        wk = ctx.enter_context(tc.tile_pool(name="work", bufs=4))

        # --- 3-stage software pipeline -------------------------------
        # A(g): input DMA, E1 scan, S1 seeds, head compaction, weighted
        #       scans wn/un, t3.
        # B(g): diagonal/off-diagonal assembly, reciprocal, alpha/mcp/beta.
        # C(g): dp/xb solve scans, scatter, output DMA.
        # Issuing A(g), C(g-2), B(g-1) keeps every in-order engine queue
        # stocked with ready work (the g-2 solve depends only on stage-B
        # results from the previous iteration).
        st = {}

        def prefetch(g):
            rsl = slice(g * GROUP_ROWS, (g + 1) * GROUP_ROWS)
            y_src = y_in[rsl, :].rearrange("(p j) v -> p (j v)", p=128)
            x3d = out_ext[rsl, :].rearrange("(p j) v -> p j v", p=128)
            d = {"x3d": x3d}
            y4 = io.tile([128, FD], F32, tag="y4")
            (nc.scalar if g == 4 else nc.sync).dma_start(y4[:, :], y_src)
            d["y4"] = y4
            d["y4v"] = y4[:, :].rearrange("p (j v) -> p j v", j=FUSE)
            if g >= n_groups - N_PAIR:
                # Pool pre-reduces adjacent v-pairs so the E1 scan (DVE)
                # runs at half length next iteration.
                q1 = e1p.tile([128, FD // 2], F32, tag="q1")
                nc.gpsimd.tensor_tensor(out=q1[:, :], in0=y4[:, 0::2],
                                        in1=rpairc, op=MUL)
                q2 = e1p.tile([128, FD // 2], F32, tag="q2")
                nc.gpsimd.tensor_tensor(out=q2[:, :], in0=q1[:, :],
                                        in1=y4[:, 1::2], op=ADD)
                d["q2"] = q2
            return d

        def stage_a(g, d):
            y4 = d["y4"]
            y4v = d["y4v"]
            x3d = d["x3d"]

            # S1 per system via ratio scan (E1_t = P(y*v)_t / v_t)
            if "q2" in d:
                E1 = e1p.tile([128, FD // 2], F32, tag="E1p")
                nc.vector.tensor_tensor_scan(E1[:, :], resetv2, d["q2"][:, :],
                                             0.0, op0=MUL, op1=ADD)
                e1last = E1[:, NV // 2 - 1::NV // 2]
            else:
                E1 = e1p.tile([128, FD], F32, tag="E1")
                nc.vector.tensor_tensor_scan(E1[:, :], resetv, y4[:, :], 0.0,
                                             op0=MUL, op1=ADD)
                e1last = E1[:, NV - 1::NV]
            s1x2 = wk.tile([128, FUSE], F32, tag="s1x2")
            nc.scalar.activation(s1x2[:, :], e1last, COPY,
                                 bias=0.0, scale=pw0 * vlast)
            s1xp = wk.tile([128, FUSE], F32, tag="s1xp")
            nc.scalar.activation(s1xp[:, :], e1last, COPY,
                                 bias=0.0, scale=0.5 * pu0 * vlast)

            yh = wk.tile([128, HD], F32, tag="yh")
            yhv = yh[:, :].rearrange("p (j v) -> p j v", j=FUSE)
            nc.gpsimd.tensor_copy(out=yhv[:, :, :], in_=y4v[:, :, 0:T0])
            d["yh"] = yh
            # tail passthrough: x = y beyond T0 - stream it out as soon as
            # the input tile is resident (frees y4 after stage A)
            teng = nc.scalar if g % 2 == 0 else nc.gpsimd
            teng.dma_start(x3d[:, :, T0:NV], y4v[:, :, T0:NV])

            wg1 = wk.tile([128, HD], F32, tag="wg1")
            nc.gpsimd.tensor_tensor(out=wg1[:, :], in0=yh[:, :], in1=g1wh, op=MUL)
            wg2 = wk.tile([128, HD], F32, tag="wg2")
            nc.gpsimd.tensor_tensor(out=wg2[:, :], in0=yh[:, :], in1=g2wh, op=MUL)
            nc.gpsimd.tensor_tensor(out=wg1[:, 0::T0], in0=wg1[:, 0::T0],
                                    in1=s1x2[:, :], op=ADD)
            nc.gpsimd.tensor_tensor(out=wg2[:, 0::T0], in0=wg2[:, 0::T0],
                                    in1=s1xp[:, :], op=ADD)

            wn = wk.tile([128, HD], F32, tag="wn")
            nc.vector.tensor_tensor_scan(wn[:, :], reset1h, wg1[:, :], 0.0,
                                         op0=MUL, op1=ADD)
            un = wk.tile([128, HD], F32, tag="un")
            nc.vector.tensor_tensor_scan(un[:, :], pw2kh, wg2[:, :], 0.0,
                                         op0=MUL, op1=ADD)
            d["wn"], d["un"] = wn, un

            t3 = wk.tile([128, HD], F32, tag="t3")
            nc.scalar.activation(t3[:, :], yh[:, :], COPY,
                                 bias=1.0, scale=float(8.0 * np.pi * Y_DT))
            d["t3"] = t3

        def stage_b(g, d):
            wn, un, yh, t3 = d["wn"], d["un"], d["yh"], d["t3"]
            b1 = wk.tile([128, HD], F32, tag="b1")
            nc.gpsimd.tensor_tensor(out=b1[:, :], in0=t3[:, :], in1=un[:, :], op=SUB)
            bil = wk.tile([128, HD], F32, tag="bil")
            nc.gpsimd.tensor_tensor(out=bil[:, :], in0=wn[:, :],
                                    in1=ilp[:, g * HD:(g + 1) * HD], op=MUL)
            bpos = wk.tile([128, HD], F32, tag="bpos")
            nc.gpsimd.tensor_tensor(out=bpos[:, :], in0=b1[:, :], in1=bil[:, :], op=ADD)
            binv = wk.tile([128, HD], F32, tag="binv")
            nc.vector.reciprocal(out=binv[:, :], in_=bpos[:, :])
            binv2 = wk.tile([128, HD], F32, tag="binv2")
            nc.gpsimd.tensor_tensor(out=binv2[:, :], in0=binv[:, :], in1=twos[:, :], op=MUL)

            a_n = wk.tile([128, HD], F32, tag="a_n")
            nc.gpsimd.tensor_tensor(out=a_n[:, :], in0=un[:, :], in1=wn[:, :], op=SUB)
            c_n = wk.tile([128, HD], F32, tag="c_n")
            nc.gpsimd.tensor_tensor(out=c_n[:, :], in0=un[:, :], in1=wn[:, :], op=ADD)

            alpha = wk.tile([128, HD], F32, tag="alpha")    # -a/b
            nc.gpsimd.tensor_tensor(out=alpha[:, :], in0=a_n[:, :], in1=binv2[:, :], op=MUL)
            av = alpha[:, :].rearrange("p (j v) -> p j v", j=FUSE)
            nc.gpsimd.memset(av[:, :, 0:1], 0.0)            # fwd scan reset
            mcp = wk.tile([128, HD], F32, tag="mcp")        # -c/b
            nc.gpsimd.tensor_tensor(out=mcp[:, :], in0=c_n[:, :], in1=binv2[:, :], op=MUL)
            mv = mcp[:, :].rearrange("p (j v) -> p j v", j=FUSE)
            nc.gpsimd.memset(mv[:, :, T0 - 1:T0], 0.0)      # bwd scan reset
            beta = wk.tile([128, HD], F32, tag="beta")      # y/b
            nc.gpsimd.tensor_tensor(out=beta[:, :], in0=yh[:, :], in1=binv[:, :], op=MUL)
            d["alpha"], d["mcp"], d["beta"] = alpha, mcp, beta

        def stage_c(g, d):
            alpha, mcp, beta = d["alpha"], d["mcp"], d["beta"]
            dp = wk.tile([128, HD], F32, tag="dp")
            nc.vector.tensor_tensor_scan(dp[:, :], alpha[:, :], beta[:, :], 0.0,
                                         op0=MUL, op1=ADD)
            xh = wk.tile([128, HD], F32, tag="xh")
            nc.vector.tensor_tensor_scan(xh[:, ::-1], mcp[:, ::-1], dp[:, ::-1], 0.0,
                                         op0=MUL, op1=ADD)
            xhv = xh[:, :].rearrange("p (j v) -> p j v", j=FUSE)
            nc.sync.dma_start(d["x3d"][:, :, 0:T0], xhv[:, :, :])

        st[0] = prefetch(0)
        for g in range(n_groups + 2):
            if g < n_groups:
                stage_a(g, st[g])
            if g + 1 < n_groups:
                st[g + 1] = prefetch(g + 1)
            if g - 2 >= 0:
                stage_c(g - 2, st[g - 2])
                del st[g - 2]
            if g - 1 < n_groups and g - 1 >= 0:
                stage_b(g - 1, st[g - 1])

    if legalize:
        _legalize_multiwait(nc)
    return nc


_NC_CACHE = {}


def _get_nc(n_groups=N_GROUPS):
    if n_groups not in _NC_CACHE:
        _NC_CACHE[n_groups] = build_nc(n_groups)
    return _NC_CACHE[n_groups]


_CF_CACHE = None


def make_inputs(y_shard, il2_rows, n_groups=N_GROUPS):
    """Per-core input map. y_shard [rows, 512] f32; il2_rows [rows] f32
    (holding il*(il+1)/2 per row)."""
    global _CF_CACHE
    if _CF_CACHE is None:
        _CF_CACHE = np.broadcast_to(_profiles()[None, :], (128, CF_W)
                                    ).astype(np.float32).copy()
    il2 = il2_rows.reshape(n_groups, 128, FUSE)[:, :, 0]          # [g, 128]
    prof = (4.0 * DV / _V[:T0]).astype(np.float64)                # [T0]
    ilp = (il2[:, :, None, None] * prof[None, None, None, :])     # [g,128,1,T0]
    ilp = np.broadcast_to(ilp, (n_groups, 128, FUSE, T0))
    ilp = ilp.transpose(1, 0, 2, 3).reshape(128, n_groups * HD).astype(np.float32)
    return {
        "y": np.ascontiguousarray(y_shard, dtype=np.float32),
        "cf": _CF_CACHE,
        "ilp": np.ascontiguousarray(ilp),
    }


def kernel(y, il_arr):
    y = np.asarray(y, dtype=np.float32)
    il_arr = np.asarray(il_arr)
    yf = y.reshape(ROWS_TOTAL, NV)
    il_f = il_arr.astype(np.float64)
    il2_all = np.repeat(il_f * (il_f + 1.0) / 2.0, NX * NY).astype(np.float32)

    nc = _get_nc()
    in_maps = []
    for c in range(N_CORES):
        rs = slice(c * ROWS_PER_CORE, (c + 1) * ROWS_PER_CORE)
        in_maps.append(make_inputs(yf[rs], il2_all[rs]))
    res = run_bass_kernel_spmd(nc, in_maps, core_ids=list(range(N_CORES)))
    outs = [res.results[c]["out"] for c in range(N_CORES)]
    x = np.concatenate(outs, axis=0).reshape(N_MODES, NX, NY, NV)
    return x.astype(np.float32)
